# revision 28
# baseline (speedup 1.0000x reference)
"""MoE (8 experts, top-2, capacity 1280) on 8 trn2 cores.

Phase 1: data-parallel gating (stats + 2-layer gate) -> logits per token.
Host: softmax/top-2/capacity selection + token dispatch (gather).
Phase 2: expert-parallel SwiGLU MLP, one expert per core.
"""

import numpy as np

import concourse.bacc as bacc
import concourse.mybir as mybir
import concourse.tile as tile
from concourse.bass_utils import run_bass_kernel_spmd
from concourse.masks import make_identity

B, N, C = 4, 2048, 2048
E, TOP_K = 8, 2
I = C * 8 // 3            # 5461
T = B * N                 # 8192
CAP = 1280
S = T // 8                # tokens per gating core: 1024
IP = 5504                 # I padded to 43*128
KC = C // 128             # 16 k-chunks over C
KG = 17                   # k-chunks over C + stats (2176 = 17*128)
MI = IP // 128            # 43 feature chunks per half
F32 = mybir.dt.float32

_cache = {}


# ---------------------------------------------------------------- phase 1

def build_gate():
    nc = bacc.Bacc("TRN2", target_bir_lowering=False)
    xs = nc.dram_tensor("xs", [S, C], F32, kind="ExternalInput")
    xst = nc.dram_tensor("xst", [C, S], F32, kind="ExternalInput")
    g1wt = nc.dram_tensor("g1wt", [KG * 128, 1024], F32, kind="ExternalInput")
    g1bm = nc.dram_tensor("g1bm", [128, 8], F32, kind="ExternalInput")
    g2wt = nc.dram_tensor("g2wt", [1024, 8], F32, kind="ExternalInput")
    logits_t = nc.dram_tensor("logits_t", [8, S], F32, kind="ExternalOutput")

    xst_v = xst.rearrange("(ko p) t -> p ko t", p=128)
    g1wt_v = g1wt.rearrange("(ko p) m -> p ko m", p=128)
    g2wt_v = g2wt.rearrange("(ko p) e -> p ko e", p=128)

    with tile.TileContext(nc) as tc:
        with (
            tc.tile_pool(name="const", bufs=1) as const,
            tc.tile_pool(name="big", bufs=1) as big,
            tc.tile_pool(name="xp", bufs=2) as xp,
            tc.tile_pool(name="tmp", bufs=3) as tmp,
            tc.tile_pool(name="stat", bufs=3) as statp,
            tc.tile_pool(name="w1", bufs=3) as w1p,
            tc.tile_pool(name="psum", bufs=2, space="PSUM") as psum,
        ):
            ident = const.tile([128, 128], F32)
            make_identity(nc, ident[:])
            g1b_sb = const.tile([128, 8], F32)
            nc.sync.dma_start(g1b_sb[:], g1bm[:])
            g2w_sb = const.tile([128, 8, 8], F32)
            nc.sync.dma_start(g2w_sb[:], g2wt_v[:])

            xst_sb = big.tile([128, KC, S], F32)          # 8.4 MB
            nc.sync.dma_start(xst_sb[:], xst_v[:])

            statsK = big.tile([128, S], F32)              # rows 0..5 = stats
            nc.vector.memset(statsK[:], 0.0)

            # ---- per-token stats (spread over ACT/DVE/Pool engines) ----
            def stats_tile(t):
                x_t = xp.tile([128, C], F32, name="x_t")
                nc.sync.dma_start(x_t[:], xs[t * 128:(t + 1) * 128, :])

                st = statp.tile([128, 8], F32, tag="st", name="st")
                nc.vector.memset(st[:], 0.0)

                sums = statp.tile([128, 1], F32, tag="sums", name="sums")
                nc.vector.reduce_sum(sums[:], x_t[:], axis=mybir.AxisListType.X)
                nc.vector.tensor_scalar_mul(st[:, 0:1], sums[:], 1.0 / C)
                nc.vector.tensor_reduce(st[:, 3:4], x_t[:],
                                        axis=mybir.AxisListType.X,
                                        op=mybir.AluOpType.max)
                # min via DVE
                nc.vector.tensor_reduce(st[:, 2:3], x_t[:],
                                        axis=mybir.AxisListType.X,
                                        op=mybir.AluOpType.min)

                # one ACT pass: sq = x^2, ssq = sum(x^2)
                sq = tmp.tile([128, C], F32, tag="scratch", name="sq")
                ssq = statp.tile([128, 1], F32, tag="ssq", name="ssq")
                nc.scalar.activation(sq[:], x_t[:],
                                     mybir.ActivationFunctionType.Square,
                                     accum_out=ssq[:])
                # l2 = sqrt(ssq)
                nc.scalar.sqrt(st[:, 4:5], ssq[:])
                # std = sqrt((ssq - C*mean^2) / (C-1))
                msq = statp.tile([128, 1], F32, tag="msq", name="msq")
                nc.vector.tensor_tensor(msq[:], st[:, 0:1], st[:, 0:1],
                                        op=mybir.AluOpType.mult)
                nc.vector.tensor_scalar(msq[:], msq[:], -float(C), None,
                                        op0=mybir.AluOpType.mult)
                nc.vector.tensor_add(msq[:], msq[:], ssq[:])
                nc.scalar.activation(st[:, 1:2], msq[:],
                                     mybir.ActivationFunctionType.Sqrt,
                                     scale=1.0 / (C - 1))

                # sparsity: one ACT abs pass + one DVE pass with accum
                ab = tmp.tile([128, C], F32, tag="scratch", name="ab")
                nc.scalar.activation(ab[:], x_t[:],
                                     mybir.ActivationFunctionType.Abs)
                spc = statp.tile([128, 1], F32, tag="spc", name="spc")
                nc.vector.tensor_scalar(ab[:], ab[:], 1e-6, 1.0,
                                        op0=mybir.AluOpType.is_lt,
                                        op1=mybir.AluOpType.mult,
                                        accum_out=spc[:])
                nc.vector.tensor_scalar_mul(st[:, 5:6], spc[:], 1.0 / C)

                # transpose [128, 6] -> [6, 128] into statsK
                pst = psum.tile([128, 128], F32, tag="pst", name="pst")
                nc.tensor.transpose(pst[:6, :], st[:, :6], ident[:])
                nc.vector.tensor_copy(statsK[:6, t * 128:(t + 1) * 128], pst[:6, :])

            # ---- h = gelu(gin @ g1_w.T + b) ----  [feat, tok] layout
            # stats for token half n are emitted just before the mm over that
            # half so the second half's stats hide under the first half's mms
            h_all = big.tile([128, 8, S], F32)            # 4 MB
            for n in range(S // 512):
                for t in range(4 * n, 4 * (n + 1)):
                    stats_tile(t)
                for m in range(8):
                    w1t = w1p.tile([128, KG, 128], F32, tag="w1t", name="w1t")
                    nc.sync.dma_start(w1t[:], g1wt_v[:, :, m * 128:(m + 1) * 128])
                    ps = psum.tile([128, 512], F32, tag="ps1", name="ps1")
                    for k in range(KG):
                        rhs = (xst_sb[:, k, n * 512:(n + 1) * 512] if k < KC
                               else statsK[:, n * 512:(n + 1) * 512])
                        nc.tensor.matmul(ps[:], w1t[:, k, :], rhs,
                                         start=(k == 0), stop=(k == KG - 1))
                    nc.scalar.activation(h_all[:, m, n * 512:(n + 1) * 512], ps[:],
                                         mybir.ActivationFunctionType.Gelu,
                                         bias=g1b_sb[:, m:m + 1])

            # ---- logits = h @ g2_w.T ----  [expert, tok] layout
            lg = big.tile([8, S], F32)
            for n in range(S // 512):
                ps2 = psum.tile([8, 512], F32, tag="ps2")
                for m in range(8):
                    nc.tensor.matmul(ps2[:], g2w_sb[:, m, :],
                                     h_all[:, m, n * 512:(n + 1) * 512],
                                     start=(m == 0), stop=(m == 7))
                nc.vector.tensor_copy(lg[:, n * 512:(n + 1) * 512], ps2[:])
            nc.sync.dma_start(logits_t[:], lg[:])
    nc.finalize()
    return nc


# ---------------------------------------------------------------- phase 2

def build_expert():
    MMDT = mybir.dt.float32r   # reduced-precision fp32 matmul (~11 mantissa
    #                            bits on the multipliers, fp32 accumulate)
    nc = bacc.Bacc("TRN2", target_bir_lowering=False)
    xet = nc.dram_tensor("xet", [C, CAP], MMDT, kind="ExternalInput")
    w1gt = nc.dram_tensor("w1gt", [C, IP], MMDT, kind="ExternalInput")
    w1ut = nc.dram_tensor("w1ut", [C, IP], MMDT, kind="ExternalInput")
    w2t = nc.dram_tensor("w2t", [IP, 2048], MMDT, kind="ExternalInput")
    kwm = nc.dram_tensor("kwm", [128, CAP // 128], F32, kind="ExternalInput")
    oute = nc.dram_tensor("oute", [CAP, C], F32, kind="ExternalOutput")

    xet_v = xet.rearrange("(ko p) t -> p ko t", p=128)
    w1gt_v = w1gt.rearrange("(ko p) m -> p ko m", p=128)
    w1ut_v = w1ut.rearrange("(ko p) m -> p ko m", p=128)
    w2t_v = w2t.rearrange("(ko p) n -> p ko n", p=128)

    TB = [(0, 512), (512, 512), (1024, 256)]   # token blocks for mm1 (psum cap)
    TB2 = [(0, 640), (640, 640)]               # token groups for mm2

    with tile.TileContext(nc) as tc:
        with (
            tc.tile_pool(name="dram", bufs=1, space="DRAM") as dpool,
            tc.tile_pool(name="const", bufs=1) as const,
        ):
            kw_sb = const.tile([128, CAP // 128], F32)
            nc.sync.dma_start(kw_sb[:], kwm[:])

            act_dram = dpool.tile([128, MI, CAP], MMDT)   # 28 MB DRAM scratch

            # ---- mm1: hh = xe @ w1.T ; act = silu(hh_g) * hh_u ----
            with (
                tc.tile_pool(name="xe", bufs=1) as xep,
                tc.tile_pool(name="w1", bufs=4) as w1p,
                tc.tile_pool(name="act", bufs=6) as actp,
                tc.tile_pool(name="ps1", bufs=2, space="PSUM") as psum1,
            ):
                xe_sb = xep.tile([128, KC, CAP], MMDT)    # 10.5 MB
                for k in range(KC):
                    nc.sync.dma_start(xe_sb[:, k, :], xet_v[:, k, :])
                for mi in range(MI):
                    wg = w1p.tile([128, KC, 128], MMDT, tag="wg")
                    nc.sync.dma_start(wg[:], w1gt_v[:, :, mi * 128:(mi + 1) * 128])
                    wu = w1p.tile([128, KC, 128], MMDT, tag="wu")
                    nc.sync.dma_start(wu[:], w1ut_v[:, :, mi * 128:(mi + 1) * 128])
                    for (t0, tw) in TB:
                        pg = psum1.tile([128, 512], F32, tag="pg")
                        pu = psum1.tile([128, 512], F32, tag="pu")
                        for k in range(KC):
                            nc.tensor.matmul(pg[:, :tw], wg[:, k, :],
                                             xe_sb[:, k, t0:t0 + tw],
                                             start=(k == 0), stop=(k == KC - 1))
                        for k in range(KC):
                            nc.tensor.matmul(pu[:, :tw], wu[:, k, :],
                                             xe_sb[:, k, t0:t0 + tw],
                                             start=(k == 0), stop=(k == KC - 1))
                        sl = actp.tile([128, 512], F32, tag="sl")
                        nc.scalar.activation(sl[:, :tw], pg[:, :tw],
                                             mybir.ActivationFunctionType.Silu)
                        ac = actp.tile([128, 512], MMDT, tag="ac")
                        nc.vector.tensor_mul(ac[:, :tw], sl[:, :tw], pu[:, :tw])
                        nc.sync.dma_start(act_dram[:, mi, t0:t0 + tw], ac[:, :tw])

            # ---- mm2: out = act @ w2.T, scaled by keep_w ----
            with (
                tc.tile_pool(name="atg", bufs=1) as atgp,
                tc.tile_pool(name="w2", bufs=6) as w2p,
                tc.tile_pool(name="out", bufs=6) as outp,
                tc.tile_pool(name="ps2", bufs=1, space="PSUM") as psum2,
            ):
                for gi, (t0, tw) in enumerate(TB2):
                    atg = atgp.tile([128, MI, 640], MMDT, tag="atg")
                    for k in range(MI):
                        nc.sync.dma_start(atg[:, k, :tw],
                                          act_dram[:, k, t0:t0 + tw])
                    nts = tw // 128
                    for n in range(4):
                        pos = [psum2.tile([128, 512], F32, tag=f"po{ts}",
                                          name=f"po{ts}")
                               for ts in range(nts)]
                        for k in range(MI):
                            w2tile = w2p.tile([128, 512], MMDT, tag="w2tile")
                            nc.sync.dma_start(w2tile[:],
                                              w2t_v[:, k, n * 512:(n + 1) * 512])
                            for ts in range(nts):
                                nc.tensor.matmul(pos[ts][:],
                                                 atg[:, k, ts * 128:(ts + 1) * 128],
                                                 w2tile[:],
                                                 start=(k == 0), stop=(k == MI - 1))
                        for ts in range(nts):
                            col = t0 // 128 + ts
                            ot = outp.tile([128, 512], F32, tag="ot")
                            nc.vector.tensor_mul(
                                ot[:], pos[ts][:],
                                kw_sb[:, col:col + 1].to_broadcast([128, 512]))
                            nc.sync.dma_start(
                                oute[t0 + ts * 128:t0 + (ts + 1) * 128,
                                     n * 512:(n + 1) * 512], ot[:])
    nc.finalize()
    return nc


# ---------------------------------------------------------------- host glue

def _get(name, fn):
    if name not in _cache:
        _cache[name] = fn()
    return _cache[name]


def _prep_gate_inputs(tokens, g1_w, g1_b, g2_w):
    key = "gate_inputs"
    if key in _cache:
        g1wt, g1bm, g2wt = _cache[key]
    else:
        g1wt = np.zeros((KG * 128, 1024), np.float32)
        g1wt[:C + 6] = np.ascontiguousarray(g1_w.T)
        g1bm = np.ascontiguousarray(g1_b.reshape(8, 128).T)
        g2wt = np.ascontiguousarray(g2_w.T)
        _cache[key] = (g1wt, g1bm, g2wt)
    tokens_t = np.ascontiguousarray(tokens.T)   # (C, T)
    in_maps = []
    for c in range(8):
        in_maps.append({
            "xs": tokens[c * S:(c + 1) * S],
            "xst": np.ascontiguousarray(tokens_t[:, c * S:(c + 1) * S]),
            "g1wt": g1wt, "g1bm": g1bm, "g2wt": g2wt,
        })
    return in_maps


def _prep_expert_weights(w1, w2):
    key = "expert_weights"
    if key not in _cache:
        per_core = []
        for e in range(E):
            w1gt = np.zeros((C, IP), np.float32)
            w1gt[:, :I] = w1[e, :I].T
            w1ut = np.zeros((C, IP), np.float32)
            w1ut[:, :I] = w1[e, I:].T
            w2te = np.zeros((IP, 2048), np.float32)
            w2te[:I] = w2[e].T
            per_core.append((w1gt, w1ut, w2te))
        _cache[key] = per_core
    return _cache[key]


def _route(logits):
    """Mirror the reference routing semantics exactly (in float32)."""
    m = logits.max(-1, keepdims=True)
    ex = np.exp(logits - m)
    probs = ex / ex.sum(-1, keepdims=True)
    ik = np.argsort(-probs, axis=-1, kind="stable")[:, :TOP_K].astype(np.int32)
    pk = np.take_along_axis(probs, ik, -1)
    pk = pk / pk.sum(-1, keepdims=True)
    toks, kws = [], []
    for e in range(E):
        wslot = np.where(ik == e, pk, -1.0).reshape(-1).astype(np.float32)
        pos = np.argsort(-wslot, kind="stable")[:CAP]
        vals = wslot[pos]
        kws.append(np.where(vals > 0.0, vals, 0.0).astype(np.float32))
        toks.append((pos // TOP_K).astype(np.int64))
    return toks, kws


def kernel(x, t, g1_w, g1_b, g2_w, w1, w2):
    x = np.asarray(x, np.float32)
    tokens = np.ascontiguousarray(x.reshape(T, C))
    nc1 = _get("nc_gate", build_gate)
    in1 = _prep_gate_inputs(tokens, np.asarray(g1_w, np.float32),
                            np.asarray(g1_b, np.float32),
                            np.asarray(g2_w, np.float32))
    r1 = run_bass_kernel_spmd(nc1, in1, list(range(8)))
    logits = np.concatenate(
        [r1.results[c]["logits_t"].T for c in range(8)], axis=0)  # (T, E)

    toks, kws = _route(logits)

    ew = _prep_expert_weights(np.asarray(w1, np.float32),
                              np.asarray(w2, np.float32))
    nc2 = _get("nc_expert", build_expert)
    in2 = []
    for e in range(E):
        w1gt, w1ut, w2te = ew[e]
        xet = np.ascontiguousarray(tokens[toks[e]].T)
        kwm = np.ascontiguousarray(kws[e].reshape(CAP // 128, 128).T)
        in2.append({"xet": xet, "w1gt": w1gt, "w1ut": w1ut, "w2t": w2te,
                    "kwm": kwm})
    r2 = run_bass_kernel_spmd(nc2, in2, list(range(8)))

    y = np.zeros((T, C), np.float32)
    for e in range(E):
        keep = kws[e] > 0.0   # kept slots reference unique tokens per expert
        y[toks[e][keep]] += r2.results[e]["oute"][keep]
    lb_loss = np.zeros((), np.float32)
    return y.reshape(B, N, C), lb_loss


# revision 29
# speedup vs baseline: 24346.8443x; 24346.8443x over previous
"""MoE (8 experts, top-2, capacity 1280) on 8 trn2 cores.

Phase 1: data-parallel gating (stats + 2-layer gate) -> logits per token.
Host: softmax/top-2/capacity selection + token dispatch (gather).
Phase 2: expert-parallel SwiGLU MLP, one expert per core.
"""

import numpy as np

import concourse.bacc as bacc
import concourse.mybir as mybir
import concourse.tile as tile
from concourse.bass_utils import run_bass_kernel_spmd
from concourse.masks import make_identity

B, N, C = 4, 2048, 2048
E, TOP_K = 8, 2
I = C * 8 // 3            # 5461
T = B * N                 # 8192
CAP = 1280
S = T // 8                # tokens per gating core: 1024
IP = 5504                 # I padded to 43*128
KC = C // 128             # 16 k-chunks over C
KG = 17                   # k-chunks over C + stats (2176 = 17*128)
MI = IP // 128            # 43 feature chunks per half
F32 = mybir.dt.float32

_cache = {}


# ---------------------------------------------------------------- phase 1

def build_gate():
    nc = bacc.Bacc("TRN2", target_bir_lowering=False)
    xs = nc.dram_tensor("xs", [S, C], F32, kind="ExternalInput")
    xst = nc.dram_tensor("xst", [C, S], F32, kind="ExternalInput")
    g1wt = nc.dram_tensor("g1wt", [KG * 128, 1024], F32, kind="ExternalInput")
    g1bm = nc.dram_tensor("g1bm", [128, 8], F32, kind="ExternalInput")
    g2wt = nc.dram_tensor("g2wt", [1024, 8], F32, kind="ExternalInput")
    logits_t = nc.dram_tensor("logits_t", [8, S], F32, kind="ExternalOutput")

    xst_v = xst.rearrange("(ko p) t -> p ko t", p=128)
    g1wt_v = g1wt.rearrange("(ko p) m -> p ko m", p=128)
    g2wt_v = g2wt.rearrange("(ko p) e -> p ko e", p=128)

    with tile.TileContext(nc) as tc:
        with (
            tc.tile_pool(name="const", bufs=1) as const,
            tc.tile_pool(name="big", bufs=1) as big,
            tc.tile_pool(name="xp", bufs=2) as xp,
            tc.tile_pool(name="tmp", bufs=3) as tmp,
            tc.tile_pool(name="stat", bufs=3) as statp,
            tc.tile_pool(name="w1", bufs=3) as w1p,
            tc.tile_pool(name="psum", bufs=2, space="PSUM") as psum,
        ):
            ident = const.tile([128, 128], F32)
            make_identity(nc, ident[:])
            g1b_sb = const.tile([128, 8], F32)
            nc.sync.dma_start(g1b_sb[:], g1bm[:])
            g2w_sb = const.tile([128, 8, 8], F32)
            nc.sync.dma_start(g2w_sb[:], g2wt_v[:])

            xst_sb = big.tile([128, KC, S], F32)          # 8.4 MB
            nc.sync.dma_start(xst_sb[:], xst_v[:])

            statsK = big.tile([128, S], F32)              # rows 0..5 = stats
            nc.vector.memset(statsK[:], 0.0)

            # ---- per-token stats (spread over ACT/DVE/Pool engines) ----
            def stats_tile(t):
                x_t = xp.tile([128, C], F32, name="x_t")
                nc.sync.dma_start(x_t[:], xs[t * 128:(t + 1) * 128, :])

                st = statp.tile([128, 8], F32, tag="st", name="st")
                nc.vector.memset(st[:], 0.0)

                sums = statp.tile([128, 1], F32, tag="sums", name="sums")
                nc.vector.reduce_sum(sums[:], x_t[:], axis=mybir.AxisListType.X)
                nc.vector.tensor_scalar_mul(st[:, 0:1], sums[:], 1.0 / C)
                nc.vector.tensor_reduce(st[:, 3:4], x_t[:],
                                        axis=mybir.AxisListType.X,
                                        op=mybir.AluOpType.max)
                # min via DVE
                nc.vector.tensor_reduce(st[:, 2:3], x_t[:],
                                        axis=mybir.AxisListType.X,
                                        op=mybir.AluOpType.min)

                # one ACT pass: sq = x^2, ssq = sum(x^2)
                sq = tmp.tile([128, C], F32, tag="scratch", name="sq")
                ssq = statp.tile([128, 1], F32, tag="ssq", name="ssq")
                nc.scalar.activation(sq[:], x_t[:],
                                     mybir.ActivationFunctionType.Square,
                                     accum_out=ssq[:])
                # l2 = sqrt(ssq)
                nc.scalar.sqrt(st[:, 4:5], ssq[:])
                # std = sqrt((ssq - C*mean^2) / (C-1))
                msq = statp.tile([128, 1], F32, tag="msq", name="msq")
                nc.vector.tensor_tensor(msq[:], st[:, 0:1], st[:, 0:1],
                                        op=mybir.AluOpType.mult)
                nc.vector.tensor_scalar(msq[:], msq[:], -float(C), None,
                                        op0=mybir.AluOpType.mult)
                nc.vector.tensor_add(msq[:], msq[:], ssq[:])
                nc.scalar.activation(st[:, 1:2], msq[:],
                                     mybir.ActivationFunctionType.Sqrt,
                                     scale=1.0 / (C - 1))

                # sparsity: one ACT abs pass + one DVE pass with accum
                ab = tmp.tile([128, C], F32, tag="scratch", name="ab")
                nc.scalar.activation(ab[:], x_t[:],
                                     mybir.ActivationFunctionType.Abs)
                spc = statp.tile([128, 1], F32, tag="spc", name="spc")
                nc.vector.tensor_scalar(ab[:], ab[:], 1e-6, 1.0,
                                        op0=mybir.AluOpType.is_lt,
                                        op1=mybir.AluOpType.mult,
                                        accum_out=spc[:])
                nc.vector.tensor_scalar_mul(st[:, 5:6], spc[:], 1.0 / C)

                # transpose [128, 6] -> [6, 128] into statsK
                pst = psum.tile([128, 128], F32, tag="pst", name="pst")
                nc.tensor.transpose(pst[:6, :], st[:, :6], ident[:])
                nc.vector.tensor_copy(statsK[:6, t * 128:(t + 1) * 128], pst[:6, :])

            # ---- h = gelu(gin @ g1_w.T + b) ----  [feat, tok] layout
            # stats for token half n are emitted just before the mm over that
            # half so the second half's stats hide under the first half's mms
            h_all = big.tile([128, 8, S], F32)            # 4 MB
            for n in range(S // 512):
                for t in range(4 * n, 4 * (n + 1)):
                    stats_tile(t)
                for m in range(8):
                    w1t = w1p.tile([128, KG, 128], F32, tag="w1t", name="w1t")
                    nc.sync.dma_start(w1t[:], g1wt_v[:, :, m * 128:(m + 1) * 128])
                    ps = psum.tile([128, 512], F32, tag="ps1", name="ps1")
                    for k in range(KG):
                        rhs = (xst_sb[:, k, n * 512:(n + 1) * 512] if k < KC
                               else statsK[:, n * 512:(n + 1) * 512])
                        nc.tensor.matmul(ps[:], w1t[:, k, :], rhs,
                                         start=(k == 0), stop=(k == KG - 1))
                    nc.scalar.activation(h_all[:, m, n * 512:(n + 1) * 512], ps[:],
                                         mybir.ActivationFunctionType.Gelu,
                                         bias=g1b_sb[:, m:m + 1])

            # ---- logits = h @ g2_w.T ----  [expert, tok] layout
            lg = big.tile([8, S], F32)
            for n in range(S // 512):
                ps2 = psum.tile([8, 512], F32, tag="ps2")
                for m in range(8):
                    nc.tensor.matmul(ps2[:], g2w_sb[:, m, :],
                                     h_all[:, m, n * 512:(n + 1) * 512],
                                     start=(m == 0), stop=(m == 7))
                nc.vector.tensor_copy(lg[:, n * 512:(n + 1) * 512], ps2[:])
            nc.sync.dma_start(logits_t[:], lg[:])
    nc.finalize()
    return nc


# ---------------------------------------------------------------- phase 2

def build_expert():
    MMDT = mybir.dt.float32r   # reduced-precision fp32 matmul (~11 mantissa
    #                            bits on the multipliers, fp32 accumulate)
    nc = bacc.Bacc("TRN2", target_bir_lowering=False)
    xet = nc.dram_tensor("xet", [C, CAP], MMDT, kind="ExternalInput")
    w1gt = nc.dram_tensor("w1gt", [C, IP], MMDT, kind="ExternalInput")
    w1ut = nc.dram_tensor("w1ut", [C, IP], MMDT, kind="ExternalInput")
    w2t = nc.dram_tensor("w2t", [IP, 2048], MMDT, kind="ExternalInput")
    kwm = nc.dram_tensor("kwm", [128, CAP // 128], F32, kind="ExternalInput")
    oute = nc.dram_tensor("oute", [CAP, C], F32, kind="ExternalOutput")

    xet_v = xet.rearrange("(ko p) t -> p ko t", p=128)
    w1gt_v = w1gt.rearrange("(ko p) m -> p ko m", p=128)
    w1ut_v = w1ut.rearrange("(ko p) m -> p ko m", p=128)
    w2t_v = w2t.rearrange("(ko p) n -> p ko n", p=128)

    TB = [(0, 512), (512, 512), (1024, 256)]   # token blocks for mm1 (psum cap)
    TB2 = [(0, 640), (640, 640)]               # token groups for mm2

    with tile.TileContext(nc) as tc:
        with (
            tc.tile_pool(name="dram", bufs=1, space="DRAM") as dpool,
            tc.tile_pool(name="const", bufs=1) as const,
        ):
            kw_sb = const.tile([128, CAP // 128], F32)
            nc.sync.dma_start(kw_sb[:], kwm[:])

            act_dram = dpool.tile([128, MI, CAP], MMDT)   # 28 MB DRAM scratch

            # ---- mm1: hh = xe @ w1.T ; act = silu(hh_g) * hh_u ----
            with (
                tc.tile_pool(name="xe", bufs=1) as xep,
                tc.tile_pool(name="w1", bufs=4) as w1p,
                tc.tile_pool(name="act", bufs=6) as actp,
                tc.tile_pool(name="ps1", bufs=2, space="PSUM") as psum1,
            ):
                xe_sb = xep.tile([128, KC, CAP], MMDT)    # 10.5 MB
                for k in range(KC):
                    nc.sync.dma_start(xe_sb[:, k, :], xet_v[:, k, :])
                for mi in range(MI):
                    wg = w1p.tile([128, KC, 128], MMDT, tag="wg")
                    nc.sync.dma_start(wg[:], w1gt_v[:, :, mi * 128:(mi + 1) * 128])
                    wu = w1p.tile([128, KC, 128], MMDT, tag="wu")
                    nc.sync.dma_start(wu[:], w1ut_v[:, :, mi * 128:(mi + 1) * 128])
                    for (t0, tw) in TB:
                        pg = psum1.tile([128, 512], F32, tag="pg")
                        pu = psum1.tile([128, 512], F32, tag="pu")
                        for k in range(KC):
                            nc.tensor.matmul(pg[:, :tw], wg[:, k, :],
                                             xe_sb[:, k, t0:t0 + tw],
                                             start=(k == 0), stop=(k == KC - 1))
                        for k in range(KC):
                            nc.tensor.matmul(pu[:, :tw], wu[:, k, :],
                                             xe_sb[:, k, t0:t0 + tw],
                                             start=(k == 0), stop=(k == KC - 1))
                        sl = actp.tile([128, 512], F32, tag="sl")
                        nc.scalar.activation(sl[:, :tw], pg[:, :tw],
                                             mybir.ActivationFunctionType.Silu)
                        ac = actp.tile([128, 512], MMDT, tag="ac")
                        nc.vector.tensor_mul(ac[:, :tw], sl[:, :tw], pu[:, :tw])
                        nc.sync.dma_start(act_dram[:, mi, t0:t0 + tw], ac[:, :tw])

            # ---- mm2: out = act @ w2.T, scaled by keep_w ----
            with (
                tc.tile_pool(name="atg", bufs=1) as atgp,
                tc.tile_pool(name="w2", bufs=6) as w2p,
                tc.tile_pool(name="out", bufs=6) as outp,
                tc.tile_pool(name="ps2", bufs=1, space="PSUM") as psum2,
            ):
                for gi, (t0, tw) in enumerate(TB2):
                    atg = atgp.tile([128, MI, 640], MMDT, tag="atg")
                    for k in range(MI):
                        nc.sync.dma_start(atg[:, k, :tw],
                                          act_dram[:, k, t0:t0 + tw])
                    nts = tw // 128
                    for n in range(4):
                        pos = [psum2.tile([128, 512], F32, tag=f"po{ts}",
                                          name=f"po{ts}",
                                          bufs=(2 if ts < 3 else 1))
                               for ts in range(nts)]
                        for k in range(MI):
                            w2tile = w2p.tile([128, 512], MMDT, tag="w2tile")
                            nc.sync.dma_start(w2tile[:],
                                              w2t_v[:, k, n * 512:(n + 1) * 512])
                            for ts in range(nts):
                                nc.tensor.matmul(pos[ts][:],
                                                 atg[:, k, ts * 128:(ts + 1) * 128],
                                                 w2tile[:],
                                                 start=(k == 0), stop=(k == MI - 1))
                        for ts in range(nts):
                            col = t0 // 128 + ts
                            ot = outp.tile([128, 512], F32, tag="ot")
                            nc.vector.tensor_mul(
                                ot[:], pos[ts][:],
                                kw_sb[:, col:col + 1].to_broadcast([128, 512]))
                            nc.sync.dma_start(
                                oute[t0 + ts * 128:t0 + (ts + 1) * 128,
                                     n * 512:(n + 1) * 512], ot[:])
    nc.finalize()
    return nc


# ---------------------------------------------------------------- host glue

def _get(name, fn):
    if name not in _cache:
        _cache[name] = fn()
    return _cache[name]


def _prep_gate_inputs(tokens, g1_w, g1_b, g2_w):
    key = "gate_inputs"
    if key in _cache:
        g1wt, g1bm, g2wt = _cache[key]
    else:
        g1wt = np.zeros((KG * 128, 1024), np.float32)
        g1wt[:C + 6] = np.ascontiguousarray(g1_w.T)
        g1bm = np.ascontiguousarray(g1_b.reshape(8, 128).T)
        g2wt = np.ascontiguousarray(g2_w.T)
        _cache[key] = (g1wt, g1bm, g2wt)
    tokens_t = np.ascontiguousarray(tokens.T)   # (C, T)
    in_maps = []
    for c in range(8):
        in_maps.append({
            "xs": tokens[c * S:(c + 1) * S],
            "xst": np.ascontiguousarray(tokens_t[:, c * S:(c + 1) * S]),
            "g1wt": g1wt, "g1bm": g1bm, "g2wt": g2wt,
        })
    return in_maps


def _prep_expert_weights(w1, w2):
    key = "expert_weights"
    if key not in _cache:
        per_core = []
        for e in range(E):
            w1gt = np.zeros((C, IP), np.float32)
            w1gt[:, :I] = w1[e, :I].T
            w1ut = np.zeros((C, IP), np.float32)
            w1ut[:, :I] = w1[e, I:].T
            w2te = np.zeros((IP, 2048), np.float32)
            w2te[:I] = w2[e].T
            per_core.append((w1gt, w1ut, w2te))
        _cache[key] = per_core
    return _cache[key]


def _route(logits):
    """Mirror the reference routing semantics exactly (in float32)."""
    m = logits.max(-1, keepdims=True)
    ex = np.exp(logits - m)
    probs = ex / ex.sum(-1, keepdims=True)
    ik = np.argsort(-probs, axis=-1, kind="stable")[:, :TOP_K].astype(np.int32)
    pk = np.take_along_axis(probs, ik, -1)
    pk = pk / pk.sum(-1, keepdims=True)
    toks, kws = [], []
    for e in range(E):
        wslot = np.where(ik == e, pk, -1.0).reshape(-1).astype(np.float32)
        pos = np.argsort(-wslot, kind="stable")[:CAP]
        vals = wslot[pos]
        kws.append(np.where(vals > 0.0, vals, 0.0).astype(np.float32))
        toks.append((pos // TOP_K).astype(np.int64))
    return toks, kws


def kernel(x, t, g1_w, g1_b, g2_w, w1, w2):
    x = np.asarray(x, np.float32)
    tokens = np.ascontiguousarray(x.reshape(T, C))
    nc1 = _get("nc_gate", build_gate)
    in1 = _prep_gate_inputs(tokens, np.asarray(g1_w, np.float32),
                            np.asarray(g1_b, np.float32),
                            np.asarray(g2_w, np.float32))
    r1 = run_bass_kernel_spmd(nc1, in1, list(range(8)))
    logits = np.concatenate(
        [r1.results[c]["logits_t"].T for c in range(8)], axis=0)  # (T, E)

    toks, kws = _route(logits)

    ew = _prep_expert_weights(np.asarray(w1, np.float32),
                              np.asarray(w2, np.float32))
    nc2 = _get("nc_expert", build_expert)
    in2 = []
    for e in range(E):
        w1gt, w1ut, w2te = ew[e]
        xet = np.ascontiguousarray(tokens[toks[e]].T)
        kwm = np.ascontiguousarray(kws[e].reshape(CAP // 128, 128).T)
        in2.append({"xet": xet, "w1gt": w1gt, "w1ut": w1ut, "w2t": w2te,
                    "kwm": kwm})
    r2 = run_bass_kernel_spmd(nc2, in2, list(range(8)))

    y = np.zeros((T, C), np.float32)
    for e in range(E):
        keep = kws[e] > 0.0   # kept slots reference unique tokens per expert
        y[toks[e][keep]] += r2.results[e]["oute"][keep]
    lb_loss = np.zeros((), np.float32)
    return y.reshape(B, N, C), lb_loss
